# revision 8
# baseline (speedup 1.0000x reference)
"""AttnBlock++ Trainium2 kernel (self-contained).

Problem (hardcoded): x (2,256,64,64) f32; GroupNorm(32 groups) -> 3x NIN
(1x1 conv C=256->256) -> 4-head attention over 64x64=4096 pixels per
(batch, head) -> NIN -> (x + h)/sqrt(2).

Sharding: 8 cores = 8 (batch, head) pairs (B=2 x nh=4). Each core:
  - GroupNorm of its batch's x (redundant across the 4 cores of a batch)
  - Q,K head projections [64, 4096] and V'^T [4096, 64+1] (ones column)
  - flash attention, i-outer: S^T tiles [j=128, i=512] on PSUM,
    exp (scale=1/8 fused) on ScalarE -> P bf16, U[65,512] accumulated over j
    (row 64 = softmax denominator via the ones column of V'^T)
  - final NIN W3-slice -> partial [256, 4096], divided by denominator
Host: sums the 4 per-head partials per batch, adds x and b3, / sqrt(2).
"""

import contextlib

import numpy as np
import ml_dtypes

import concourse.bass as bass
import concourse.mybir as mybir
import concourse.tile as tile
from concourse.vector_clock import ScopedClock
from concourse import bass_utils

# ---- problem constants ----
B, C, H, W = 2, 256, 64, 64
NPIX = H * W            # 4096
NH = 4                  # heads
CH = C // NH            # 64
NG = 32                 # groupnorm groups
GSZ = C // NG           # 8 channels per group
EPS = 1e-6
NCORES = 8
P = 128
NCT = C // P            # 2 channel tiles
NJ = NPIX // P          # 32 key-pixel chunks
NI = 8                  # query chunks
IW = NPIX // NI         # 512
GS = 3                  # j-chunks per exp group
ATT_SCALE = CH ** (-0.5)  # 0.125

F32 = mybir.dt.float32
BF16 = mybir.dt.bfloat16

_drain_patched = False


def patch_drain():
    """Split the TileContext exit-drain's semaphore waits across nops.

    The staged walrus build rejects a Drain instruction carrying more than
    one or two sync waits ("Too many sync wait commands"), so carry each
    wait on its own SP nop before the drain.
    """
    global _drain_patched
    if _drain_patched:
        return
    _drain_patched = True

    def _patched(self, tick_clock, wait_clock):
        carrier = self.nc.sync.nop(nofuse=True, hint="drain_wait_carrier")
        wait_clock.add_sem_waits(
            carrier.ins, ScopedClock({None: tick_clock.global_clock})
        )
        si = carrier.ins.sync_info
        waits = list(si.on_wait or [])
        if len(waits) > 1:
            si.on_wait = [waits[0]]
            for extra in waits[1:]:
                n2 = self.nc.sync.nop(nofuse=True, hint="drain_wait_extra")
                if n2.ins.sync_info is None:
                    n2.ins.sync_info = mybir.SyncInfo(on_wait=[extra], on_update=[])
                else:
                    n2.ins.sync_info.on_wait = [extra]
        self.nc.sync.drain()
        self.nc.all_engine_barrier()
        assert self.sems is not None
        popped = self.nc._tile_sem_poison_stack.pop()
        assert popped is self._sem_poison
        self.nc.clear_and_free_semaphores(list(self.sems.allocated().values()))
        self.nc.all_engine_barrier()

    tile.TileContext._drain_and_barrier = _patched


MAX_WAITS = 1  # staged walrus rejects >1 sync wait per instruction


def split_waits(nc):
    """Post-scheduling pass: hoist excess sync waits onto preceding nops.

    The staged walrus build rejects instructions carrying more than
    MAX_WAITS sem waits ("Too many sync wait commands"); a nop on the same
    engine executing immediately before is semantically equivalent.
    """
    for f in nc.m.functions:
        for bb in f.blocks:
            new_insts = []
            for inst in bb.instructions:
                si = inst.sync_info
                waits = list(si.on_wait or []) if si else []
                if len(waits) > MAX_WAITS:
                    keep = waits[:MAX_WAITS]
                    extra = waits[MAX_WAITS:]
                    for w in extra:
                        nop = mybir.InstNoOp(
                            name=nc.get_next_instruction_name(),
                            ins=[],
                            outs=[],
                        )
                        nop.engine = inst.engine
                        nop.sync_info = mybir.SyncInfo(on_wait=[w], on_update=[])
                        new_insts.append(nop)
                    si.on_wait = keep
                new_insts.append(inst)
            bb.instructions[:] = new_insts


def build_nc(repeat=1):
    """Build the SPMD per-core module. repeat>1 re-emits the whole body N
    times back-to-back (for wall-clock benchmarking by deltas)."""
    patch_drain()
    nc = bass.Bass()

    # ---- DRAM I/O (per-core views; SPMD over 8 cores) ----
    x_d = nc.dram_tensor("x", [NCT, P, NPIX], F32, kind="ExternalInput")
    gnsc_d = nc.dram_tensor("gnsc", [NCT, P, 1], F32, kind="ExternalInput")
    gnbi_d = nc.dram_tensor("gnbi", [NCT, P, 1], F32, kind="ExternalInput")
    gmask_d = nc.dram_tensor("gmask", [P, NCT, NG], F32, kind="ExternalInput")
    w0_d = nc.dram_tensor("w0h", [P, NCT, CH], BF16, kind="ExternalInput")
    w1_d = nc.dram_tensor("w1h", [P, NCT, CH], BF16, kind="ExternalInput")
    w2_d = nc.dram_tensor("w2h", [P, NCT, CH], BF16, kind="ExternalInput")
    w3_d = nc.dram_tensor("w3h", [CH, NCT, P], BF16, kind="ExternalInput")
    b0_d = nc.dram_tensor("b0h", [CH, 1], F32, kind="ExternalInput")
    b1_d = nc.dram_tensor("b1h", [CH, 1], F32, kind="ExternalInput")
    b2_d = nc.dram_tensor("b2h", [1, CH], F32, kind="ExternalInput")
    out_d = nc.dram_tensor("out", [NCT, P, NPIX], F32, kind="ExternalOutput")

    with tile.TileContext(nc) as tc, contextlib.ExitStack() as ctx:
        singles = ctx.enter_context(tc.tile_pool(name="singles", bufs=1))
        xp = ctx.enter_context(tc.tile_pool(name="xp", bufs=2))
        hp = ctx.enter_context(tc.tile_pool(name="hp", bufs=2))
        qkv = ctx.enter_context(tc.tile_pool(name="qkv", bufs=1))
        stat = ctx.enter_context(tc.tile_pool(name="stat", bufs=2))
        pP = ctx.enter_context(tc.tile_pool(name="pP", bufs=3))
        misc = ctx.enter_context(tc.tile_pool(name="misc", bufs=2))
        outp = ctx.enter_context(tc.tile_pool(name="outp", bufs=4))
        dscr = ctx.enter_context(tc.tile_pool(name="dscr", bufs=2, space="DRAM"))
        ps_S = ctx.enter_context(tc.tile_pool(name="ps_S", bufs=2, space="PSUM"))
        ps_U = ctx.enter_context(tc.tile_pool(name="ps_U", bufs=2, space="PSUM"))

        # ---- load constants / weights (once) ----
        gmask_sb = singles.tile([P, NCT, NG], F32, name="gmask_sb")
        nc.gpsimd.dma_start(out=gmask_sb, in_=gmask_d[:, :, :])
        w0_sb = singles.tile([P, NCT, CH], BF16, name="w0_sb")
        nc.gpsimd.dma_start(out=w0_sb, in_=w0_d[:, :, :])
        w1_sb = singles.tile([P, NCT, CH], BF16, name="w1_sb")
        nc.gpsimd.dma_start(out=w1_sb, in_=w1_d[:, :, :])
        w2_sb = singles.tile([P, NCT, CH], BF16, name="w2_sb")
        nc.gpsimd.dma_start(out=w2_sb, in_=w2_d[:, :, :])
        w3_sb = singles.tile([CH, NCT, P], BF16, name="w3_sb")
        nc.gpsimd.dma_start(out=w3_sb, in_=w3_d[:, :, :])
        b0_sb = singles.tile([CH, 1], F32, name="b0_sb")
        nc.gpsimd.dma_start(out=b0_sb, in_=b0_d[:, :])
        b1_sb = singles.tile([CH, 1], F32, name="b1_sb")
        nc.gpsimd.dma_start(out=b1_sb, in_=b1_d[:, :])
        b2b_sb = singles.tile([P, CH], F32, name="b2b_sb")
        nc.gpsimd.dma_start(out=b2b_sb, in_=b2_d[:, :].to_broadcast([P, CH]))
        sc_sb = singles.tile([P, NCT], F32, name="sc_sb")
        bi_sb = singles.tile([P, NCT], F32, name="bi_sb")
        for t in range(NCT):
            nc.gpsimd.dma_start(out=sc_sb[:, t : t + 1], in_=gnsc_d[t])
            nc.gpsimd.dma_start(out=bi_sb[:, t : t + 1], in_=gnbi_d[t])
        ones328 = singles.tile([NG, GSZ], F32, name="ones328")
        nc.vector.memset(ones328, 1.0)

        consts = dict(
            gmask_sb=gmask_sb, w0_sb=w0_sb, w1_sb=w1_sb, w2_sb=w2_sb,
            w3_sb=w3_sb, b0_sb=b0_sb, b1_sb=b1_sb, b2b_sb=b2b_sb,
            sc_sb=sc_sb, bi_sb=bi_sb, ones328=ones328,
        )
        pools = dict(
            xp=xp, hp=hp, qkv=qkv, stat=stat, pP=pP, misc=misc, outp=outp,
            dscr=dscr, ps_S=ps_S, ps_U=ps_U,
        )
        for rep in range(repeat):
            _emit_body(nc, x_d, out_d, consts, pools, pfx=f"r{rep}_")

    split_waits(nc)
    return nc


def _emit_body(nc, x_d, out_d, cs, pl, pfx):
    xp, hp, qkv, stat, pP, misc, outp, dscr, ps_S, ps_U = (
        pl["xp"], pl["hp"], pl["qkv"], pl["stat"], pl["pP"], pl["misc"],
        pl["outp"], pl["dscr"], pl["ps_S"], pl["ps_U"],
    )

    # ---- GroupNorm stats ----
    x_sb = []
    mcols = []
    for t in range(NCT):
        xt = xp.tile([P, NPIX], F32, tag="x", name=f"{pfx}x_{t}")
        x_sb.append(xt)
        for cc in range(4):
            nc.sync.dma_start(
                out=xt[:, cc * 1024 : (cc + 1) * 1024],
                in_=x_d[t, :, cc * 1024 : (cc + 1) * 1024],
            )
        stats = stat.tile([P, 8, 6], F32, tag="bnst", name=f"{pfx}bnst_{t}")
        for s in range(8):
            nc.vector.bn_stats(out=stats[:, s, :], in_=xt[:, s * 512 : (s + 1) * 512])
        mv = stat.tile([P, 2], F32, tag="mv", name=f"{pfx}mv_{t}")
        nc.vector.bn_aggr(out=mv, in_=stats)
        mc = stat.tile([P, 3], F32, tag="mcols", name=f"{pfx}mcols_{t}")
        nc.vector.tensor_copy(out=mc[:, 0:2], in_=mv)
        nc.vector.tensor_mul(out=mc[:, 2:3], in0=mv[:, 0:1], in1=mv[:, 0:1])
        mcols.append(mc)

    sg_ps = ps_U.tile([NG, 3], F32, tag="U", name=f"{pfx}sg_ps")
    for t in range(NCT):
        nc.tensor.matmul(
            sg_ps, lhsT=cs["gmask_sb"][:, t, :], rhs=mcols[t],
            start=(t == 0), stop=(t == NCT - 1),
        )
    sg_sb = stat.tile([NG, 3], F32, tag="sg_sb", name=f"{pfx}sg_sb")
    nc.vector.tensor_copy(out=sg_sb, in_=sg_ps)
    gm = stat.tile([NG, 1], F32, tag="gm", name=f"{pfx}gm")
    nc.vector.tensor_scalar(
        out=gm, in0=sg_sb[:, 0:1], scalar1=1.0 / GSZ, scalar2=None,
        op0=mybir.AluOpType.mult,
    )
    ex2 = stat.tile([NG, 1], F32, tag="ex2", name=f"{pfx}ex2")
    nc.vector.tensor_add(out=ex2, in0=sg_sb[:, 1:2], in1=sg_sb[:, 2:3])
    nc.vector.tensor_scalar(
        out=ex2, in0=ex2, scalar1=1.0 / GSZ, scalar2=None, op0=mybir.AluOpType.mult,
    )
    gv = stat.tile([NG, 1], F32, tag="gv", name=f"{pfx}gv")
    nc.vector.tensor_mul(out=gv, in0=gm, in1=gm)
    nc.vector.tensor_sub(out=gv, in0=ex2, in1=gv)
    nc.vector.tensor_scalar(
        out=gv, in0=gv, scalar1=float(EPS), scalar2=None, op0=mybir.AluOpType.add,
    )
    # rstd = 1/sqrt(gv), one Newton step for accuracy
    sd = stat.tile([NG, 1], F32, tag="sd", name=f"{pfx}sd")
    nc.scalar.activation(out=sd, in_=gv, func=mybir.ActivationFunctionType.Sqrt)
    y0 = stat.tile([NG, 1], F32, tag="y0", name=f"{pfx}y0")
    nc.vector.reciprocal(out=y0, in_=sd)
    tnr = stat.tile([NG, 1], F32, tag="tnr", name=f"{pfx}tnr")
    nc.vector.tensor_mul(out=tnr, in0=gv, in1=y0)
    nc.vector.tensor_mul(out=tnr, in0=tnr, in1=y0)
    nc.vector.tensor_scalar(
        out=tnr, in0=tnr, scalar1=-0.5, scalar2=1.5,
        op0=mybir.AluOpType.mult, op1=mybir.AluOpType.add,
    )
    nc.vector.tensor_mul(out=y0, in0=y0, in1=tnr)

    # broadcast group stats to channels via DRAM bounce
    m_rep = stat.tile([NG, GSZ], F32, tag="m_rep", name=f"{pfx}m_rep")
    nc.vector.tensor_scalar(
        out=m_rep, in0=cs["ones328"], scalar1=gm, scalar2=None,
        op0=mybir.AluOpType.mult,
    )
    r_rep = stat.tile([NG, GSZ], F32, tag="r_rep", name=f"{pfx}r_rep")
    nc.vector.tensor_scalar(
        out=r_rep, in0=cs["ones328"], scalar1=y0, scalar2=None,
        op0=mybir.AluOpType.mult,
    )
    m_dt = dscr.tile([NG, GSZ], F32, tag="m_dt", name=f"{pfx}m_dt")
    r_dt = dscr.tile([NG, GSZ], F32, tag="r_dt", name=f"{pfx}r_dt")
    nc.gpsimd.dma_start(out=m_dt, in_=m_rep)
    nc.gpsimd.dma_start(out=r_dt, in_=r_rep)

    h_sb = []
    for t in range(NCT):
        m_ch = stat.tile([P, 1], F32, tag="m_ch", name=f"{pfx}m_ch_{t}")
        nc.gpsimd.dma_start(
            out=m_ch,
            in_=m_dt.rearrange("g e -> (g e)").rearrange(
                "(t p) -> t p", t=NCT
            )[t].unsqueeze(1),
        )
        r_ch = stat.tile([P, 1], F32, tag="r_ch", name=f"{pfx}r_ch_{t}")
        nc.gpsimd.dma_start(
            out=r_ch,
            in_=r_dt.rearrange("g e -> (g e)").rearrange(
                "(t p) -> t p", t=NCT
            )[t].unsqueeze(1),
        )
        a_c = stat.tile([P, 1], F32, tag="a_c", name=f"{pfx}a_c_{t}")
        nc.vector.tensor_mul(out=a_c, in0=r_ch, in1=cs["sc_sb"][:, t : t + 1])
        b_c = stat.tile([P, 1], F32, tag="b_c", name=f"{pfx}b_c_{t}")
        nc.vector.tensor_mul(out=b_c, in0=m_ch, in1=a_c)
        nc.vector.tensor_sub(out=b_c, in0=cs["bi_sb"][:, t : t + 1], in1=b_c)
        ht = hp.tile([P, NPIX], BF16, tag="h", name=f"{pfx}h_{t}")
        nc.vector.tensor_scalar(
            out=ht, in0=x_sb[t], scalar1=a_c, scalar2=b_c,
            op0=mybir.AluOpType.mult, op1=mybir.AluOpType.add,
        )
        h_sb.append(ht)

    # ---- Q, K projections: [CH, NPIX] bf16 ----
    q_sb = qkv.tile([CH, NPIX], BF16, tag="q", name=f"{pfx}q_sb")
    k_sb = qkv.tile([CH, NPIX], BF16, tag="k", name=f"{pfx}k_sb")
    for dst, wname, bname in ((q_sb, "w0_sb", "b0_sb"), (k_sb, "w1_sb", "b1_sb")):
        for i in range(NI):
            ps = ps_U.tile([CH, IW], F32, tag="U", name=f"{pfx}{wname}_ps_{i}")
            for t in range(NCT):
                nc.tensor.matmul(
                    ps, lhsT=cs[wname][:, t, :],
                    rhs=h_sb[t][:, i * IW : (i + 1) * IW],
                    start=(t == 0), stop=(t == NCT - 1),
                )
            nc.vector.tensor_scalar(
                out=dst[:, i * IW : (i + 1) * IW], in0=ps,
                scalar1=cs[bname], scalar2=None, op0=mybir.AluOpType.add,
            )

    # ---- V'^T: [128, NJ, CH+1] bf16 (last col = 1.0) ----
    vt_sb = qkv.tile([P, NJ, CH + 1], BF16, tag="vt", name=f"{pfx}vt_sb")
    nc.vector.memset(vt_sb[:, :, CH : CH + 1], 1.0)
    for n in range(NJ):
        vps = ps_U.tile([P, CH], F32, tag="U", name=f"{pfx}v_ps_{n}")
        for t in range(NCT):
            nc.tensor.matmul(
                vps, lhsT=h_sb[t][:, n * P : (n + 1) * P], rhs=cs["w2_sb"][:, t, :],
                start=(t == 0), stop=(t == NCT - 1),
            )
        nc.vector.tensor_add(out=vt_sb[:, n, 0:CH], in0=vps, in1=cs["b2b_sb"])

    # ---- attention, i-outer ----
    groups = [list(range(g, min(g + GS, NJ))) for g in range(0, NJ, GS)]
    pending = [None]

    def flush_pending():
        if pending[0] is None:
            return
        U, ptile, js = pending[0]
        for idx, j in enumerate(js):
            nc.tensor.matmul(
                U[0 : CH + 1, :], lhsT=vt_sb[:, j, :], rhs=ptile[:, idx, :],
                start=(j == 0), stop=(j == NJ - 1),
            )
        pending[0] = None

    for i in range(NI):
        U = ps_U.tile([P, IW], F32, tag="U", name=f"{pfx}U_{i}")
        for js in groups:
            S = ps_S.tile([P, GS, IW], F32, tag="S", name=f"{pfx}S_{i}_{js[0]}")
            for idx, j in enumerate(js):
                nc.tensor.matmul(
                    S[:, idx, :], lhsT=k_sb[:, j * P : (j + 1) * P],
                    rhs=q_sb[:, i * IW : (i + 1) * IW], start=True, stop=True,
                )
            ptile = pP.tile([P, GS, IW], BF16, tag="P", name=f"{pfx}P_{i}_{js[0]}")
            nc.scalar.activation(
                out=ptile[:, 0 : len(js), :], in_=S[:, 0 : len(js), :],
                func=mybir.ActivationFunctionType.Exp, scale=float(ATT_SCALE),
            )
            flush_pending()
            pending[0] = (U, ptile, js)
        flush_pending()

        # tail: denominator, broadcast, NIN, store
        rden = misc.tile([1, IW], F32, tag="rden", name=f"{pfx}rden_{i}")
        nc.vector.tensor_copy(out=rden, in_=U[CH : CH + 1, :])
        rec = misc.tile([1, IW], F32, tag="rec", name=f"{pfx}rec_{i}")
        nc.vector.reciprocal(out=rec, in_=rden)
        rec_dt = dscr.tile([1, IW], F32, tag="rec_dt", name=f"{pfx}rec_dt_{i}")
        nc.sync.dma_start(out=rec_dt, in_=rec)
        rb = misc.tile([P, IW], F32, tag="rb", name=f"{pfx}rb_{i}")
        nc.sync.dma_start(out=rb, in_=rec_dt.to_broadcast([P, IW]))
        o_sb = misc.tile([CH, IW], BF16, tag="o_sb", name=f"{pfx}o_sb_{i}")
        nc.vector.tensor_copy(out=o_sb, in_=U[0:CH, :])
        for dh in range(NCT):
            F = ps_U.tile([P, IW], F32, tag="U", name=f"{pfx}F_{i}_{dh}")
            nc.tensor.matmul(
                F, lhsT=cs["w3_sb"][:, dh, :], rhs=o_sb, start=True, stop=True
            )
            ot = outp.tile([P, IW], F32, tag="out", name=f"{pfx}ot_{i}_{dh}")
            nc.vector.tensor_mul(out=ot, in0=F, in1=rb)
            nc.sync.dma_start(out=out_d[dh, :, i * IW : (i + 1) * IW], in_=ot)


def make_gmask():
    g = np.zeros((P, NCT, NG), np.float32)
    for t in range(NCT):
        for p in range(P):
            g[p, t, 16 * t + p // GSZ] = 1.0
    return g


def make_in_maps(x, gn_scale, gn_bias, W0, b0, W1, b1, W2, b2, W3, b3):
    bf = ml_dtypes.bfloat16
    gmask = make_gmask()
    gnsc = np.ascontiguousarray(gn_scale.reshape(NCT, P, 1), np.float32)
    gnbi = np.ascontiguousarray(gn_bias.reshape(NCT, P, 1), np.float32)
    in_maps = []
    for core in range(NCORES):
        b, h = divmod(core, NH)
        sl = slice(h * CH, (h + 1) * CH)
        in_maps.append(
            {
                "x": np.ascontiguousarray(x[b].reshape(NCT, P, NPIX), np.float32),
                "gnsc": gnsc,
                "gnbi": gnbi,
                "gmask": gmask,
                "w0h": np.ascontiguousarray(
                    W0[:, sl].reshape(NCT, P, CH).transpose(1, 0, 2)
                ).astype(bf),
                "w1h": np.ascontiguousarray(
                    W1[:, sl].reshape(NCT, P, CH).transpose(1, 0, 2)
                ).astype(bf),
                "w2h": np.ascontiguousarray(
                    W2[:, sl].reshape(NCT, P, CH).transpose(1, 0, 2)
                ).astype(bf),
                "w3h": np.ascontiguousarray(W3[sl, :].reshape(CH, NCT, P)).astype(bf),
                "b0h": np.ascontiguousarray(b0[sl].reshape(CH, 1), np.float32),
                "b1h": np.ascontiguousarray(b1[sl].reshape(CH, 1), np.float32),
                "b2h": np.ascontiguousarray(b2[sl].reshape(1, CH), np.float32),
            }
        )
    return in_maps


LAST_RESULTS = None  # BassKernelResults from the most recent kernel() call


def kernel(**inputs):
    global LAST_RESULTS

    x = np.asarray(inputs["x"], np.float32)
    b3 = np.asarray(inputs["b3"], np.float32)
    in_maps = make_in_maps(
        x,
        np.asarray(inputs["gn_scale"], np.float32),
        np.asarray(inputs["gn_bias"], np.float32),
        np.asarray(inputs["W0"], np.float32),
        np.asarray(inputs["b0"], np.float32),
        np.asarray(inputs["W1"], np.float32),
        np.asarray(inputs["b1"], np.float32),
        np.asarray(inputs["W2"], np.float32),
        np.asarray(inputs["b2"], np.float32),
        np.asarray(inputs["W3"], np.float32),
        b3,
    )
    nc = build_nc()
    res = bass_utils.run_bass_kernel_spmd(nc, in_maps, core_ids=list(range(NCORES)))
    LAST_RESULTS = res
    outs = [r["out"].reshape(C, NPIX) for r in res.results]
    sq2 = np.sqrt(2.0).astype(np.float32)
    y = np.empty((B, C, NPIX), np.float32)
    for b in range(B):
        acc = outs[NH * b]
        for h in range(1, NH):
            acc = acc + outs[NH * b + h]
        y[b] = (x[b].reshape(C, NPIX) + acc + b3[:, None]) / sq2
    return y.reshape(B, C, H, W)
